# revision 4
# baseline (speedup 1.0000x reference)
"""Trainium2 Bass kernel for nn_NeuroKernel_5884105195994 (dense_mlp).

Computes: K = triu(MLP3(pairs(x))), output = K.T @ K, with
pairs[i,j] = (x[i], x[j]), MLP3 = (2->1024 relu) -> (1024->128 relu) -> (128->1).

Strategy (8 NeuronCores, SPMD):
  - Row i of the [512, 512] pair grid goes to core c = i % 8, local rank
    k = i // 8 (64 rows per core). Since only columns j >= i are needed
    (triu), row rank k computes the window [8k, 512) — identical on every
    core — and a per-core {0,1} mask input kills the j < i remainder, so
    the compiled program is core-independent (true SPMD).
  - Layer 1 has contraction dim 2, so it is NOT a matmul on device:
    h1[h, j] = relu(x_j * W1[1,h] + (x_i * W1[0,h] + b1[h])) is a single
    per-partition-scalar op (DVE tensor_scalar add+max / ACT relu-with-bias)
    over precomputed bT[h, j] = x_j * W1[1,h] tiles.
  - Layer 2 is the real work: h on partitions (8 chunks of 128), pair
    window on the free dim, accumulating W2-chunk matmuls into PSUM.
  - Rows are packed in complementary pairs {k, 64-k} so every matmul /
    elementwise op runs at free-dim 512 (window widths 512-8k and 8k).
  - Layer 3 is one [128,1]x[128,W] matmul per group; the [1, W] result is
    DMA'd into K_sb[64, 512] (row-major K block).
  - Each core computes its partial G = K_blk.T @ K_blk (contraction over
    its 64 rows, 4 matmuls) on the TensorEngine; the host sums 8 partials.
"""

import sys

if "/opt/trn_rl_repo" not in sys.path:
    sys.path.insert(0, "/opt/trn_rl_repo")

import numpy as np

N = 512
N_CORES = 8
KPC = N // N_CORES  # 64 rows per core
H1 = 1024
H2 = 128
NCHUNK = H1 // 128  # 8 h-chunks
N_DVE_CHUNKS = 6  # h1 chunks produced on VectorE; rest on ScalarE


def _groups():
    """Row groups per core: list of lists of (k, col_off, j0, w)."""
    gs = [[(0, 0, 0, 512)]]
    for m in range(1, 32):
        w_a = 512 - 8 * m
        gs.append([(m, 0, 8 * m, w_a), (64 - m, w_a, 512 - 8 * m, 8 * m)])
    gs.append([(32, 0, 256, 256)])
    return gs


GROUPS = _groups()

_cached_nc = None


def _build():
    import concourse.bass as bass
    import concourse.mybir as mybir
    import concourse.tile as tile
    from concourse import bacc

    fp32 = mybir.dt.float32
    Alu = mybir.AluOpType
    Act = mybir.ActivationFunctionType

    nc = bacc.Bacc(None, target_bir_lowering=False)

    def bcast(ap, parts):
        # Prepend a stride-0 partition dim: replicate a DRAM vector across
        # `parts` SBUF partitions in one DMA.
        return bass.AP(tensor=ap.tensor, offset=ap.offset,
                       ap=[[0, parts]] + [list(d) for d in ap.ap])

    x_d = nc.dram_tensor("x", [N], fp32, kind="ExternalInput")
    xr_d = nc.dram_tensor("xr", [KPC], fp32, kind="ExternalInput")
    mask_d = nc.dram_tensor("mask", [KPC, N], fp32, kind="ExternalInput")
    w1a_d = nc.dram_tensor("w1a", [H1], fp32, kind="ExternalInput")
    w1b_d = nc.dram_tensor("w1b", [H1], fp32, kind="ExternalInput")
    b1_d = nc.dram_tensor("b1", [H1], fp32, kind="ExternalInput")
    w2_d = nc.dram_tensor("w2", [H1, H2], fp32, kind="ExternalInput")
    b2_d = nc.dram_tensor("b2", [H2], fp32, kind="ExternalInput")
    w3_d = nc.dram_tensor("w3", [H2], fp32, kind="ExternalInput")
    b3_d = nc.dram_tensor("b3", [1], fp32, kind="ExternalInput")
    g_d = nc.dram_tensor("g", [N, N], fp32, kind="ExternalOutput")

    with tile.TileContext(nc) as tc:
        with (
            tc.tile_pool(name="const", bufs=1) as const,
            tc.tile_pool(name="h1p", bufs=3) as h1p,
            tc.tile_pool(name="h2p", bufs=3) as h2p,
            tc.tile_pool(name="kout", bufs=1) as kout,
            tc.tile_pool(name="psum", bufs=3, space="PSUM") as psum,
            tc.tile_pool(name="kpsum", bufs=2, space="PSUM") as kpsum,
        ):
            # ---- constants / precompute ----
            w1a_t = const.tile([128, NCHUNK], fp32)
            nc.sync.dma_start(w1a_t[:], w1a_d[:].rearrange("(c p) -> p c", p=128))
            w1b_t = const.tile([128, NCHUNK], fp32)
            nc.sync.dma_start(w1b_t[:], w1b_d[:].rearrange("(c p) -> p c", p=128))
            b1_t = const.tile([128, NCHUNK], fp32)
            nc.sync.dma_start(b1_t[:], b1_d[:].rearrange("(c p) -> p c", p=128))
            b2_t = const.tile([128, 1], fp32)
            nc.sync.dma_start(b2_t[:], b2_d[:].rearrange("(p one) -> p one", one=1))
            w3_t = const.tile([128, 1], fp32)
            nc.sync.dma_start(w3_t[:], w3_d[:].rearrange("(p one) -> p one", one=1))
            b3_t = const.tile([KPC, 1], fp32)
            nc.sync.dma_start(b3_t[:], bcast(b3_d[:], KPC))
            mask_t = const.tile([KPC, N], fp32)
            nc.sync.dma_start(mask_t[:], mask_d[:])
            w2_t = const.tile([128, NCHUNK, H2], fp32)
            nc.sync.dma_start(w2_t[:], w2_d[:].rearrange("(c p) k -> p c k", p=128))

            xr_b = const.tile([128, KPC], fp32)
            nc.sync.dma_start(xr_b[:], bcast(xr_d[:], 128))
            xc_b = const.tile([128, N], fp32)
            nc.sync.dma_start(xc_b[:], bcast(x_d[:], 128))

            # bias[h, k] = x_row[k] * W1[0, h] + b1[h]
            bias_t = const.tile([128, NCHUNK, KPC], fp32)
            for cc in range(NCHUNK):
                nc.vector.tensor_scalar(
                    bias_t[:, cc, :], xr_b[:],
                    w1a_t[:, cc : cc + 1], b1_t[:, cc : cc + 1],
                    Alu.mult, Alu.add,
                )
            # bT[h, j] = x_col[j] * W1[1, h]
            bT_t = const.tile([128, NCHUNK, N], fp32)
            for cc in range(NCHUNK):
                nc.vector.tensor_scalar(
                    bT_t[:, cc, :], xc_b[:], w1b_t[:, cc : cc + 1], None, Alu.mult
                )

            k_sb = kout.tile([KPC, N], fp32)
            nc.vector.memset(k_sb[:], 0.0)

            # ---- main loop over row groups ----
            for rows in GROUPS:
                wg = sum(r[3] for r in rows)
                h1 = h1p.tile([128, NCHUNK, wg], fp32, tag="h1")
                for cc in range(NCHUNK):
                    for (k, off, j0, w) in rows:
                        out_ap = h1[:, cc, off : off + w]
                        in_ap = bT_t[:, cc, j0 : j0 + w]
                        bias_ap = bias_t[:, cc, k : k + 1]
                        if cc < N_DVE_CHUNKS:
                            nc.vector.tensor_scalar(
                                out_ap, in_ap, bias_ap, 0.0, Alu.add, Alu.max
                            )
                        else:
                            nc.scalar.activation(out_ap, in_ap, Act.Relu, bias=bias_ap)
                ps = psum.tile([128, wg], fp32, tag="ps")
                for cc in range(NCHUNK):
                    nc.tensor.matmul(
                        ps[:, :wg], w2_t[:, cc, :], h1[:, cc, :wg],
                        start=(cc == 0), stop=(cc == NCHUNK - 1),
                    )
                h2 = h2p.tile([128, wg], fp32, tag="h2")
                nc.scalar.activation(h2[:], ps[:, :wg], Act.Relu, bias=b2_t[:, 0:1])
                kps = kpsum.tile([1, wg], fp32, tag="kps")
                nc.tensor.matmul(kps[0:1, :wg], w3_t[:, 0:1], h2[:, :wg],
                                 start=True, stop=True)
                kst = h2p.tile([1, wg], fp32, tag="kst")
                nc.scalar.activation(kst[0:1, :wg], kps[0:1, :wg], Act.Copy)
                for (k, off, j0, w) in rows:
                    nc.sync.dma_start(
                        k_sb[k : k + 1, j0 : j0 + w], kst[0:1, off : off + w]
                    )

            # ---- K = (K + b3) * mask, then G = K.T @ K (64-row partial) ----
            km = kout.tile([KPC, N], fp32)
            nc.vector.tensor_scalar(km[:], k_sb[:], b3_t[:, 0:1], None, Alu.add)
            km2 = kout.tile([KPC, N], fp32)
            nc.vector.tensor_tensor(km2[:], km[:], mask_t[:], Alu.mult)
            for a in range(4):
                gps = psum.tile([128, N], fp32, tag="ps")
                nc.tensor.matmul(
                    gps[:], km2[:, 128 * a : 128 * (a + 1)], km2[:],
                    start=True, stop=True,
                )
                gsb = h1p.tile([128, N], fp32, tag="gsb")
                nc.vector.tensor_copy(gsb[:], gps[:])
                nc.sync.dma_start(g_d[128 * a : 128 * (a + 1), :], gsb[:])

    nc.compile()
    return nc


def kernel(x, W1, b1, W2, b2, W3, b3):
    global _cached_nc
    if _cached_nc is None:
        _cached_nc = _build()
    nc = _cached_nc

    from concourse.bass_utils import run_bass_kernel_spmd

    x = np.asarray(x, dtype=np.float32)
    W1 = np.asarray(W1, dtype=np.float32)
    b1 = np.asarray(b1, dtype=np.float32)
    W2 = np.asarray(W2, dtype=np.float32)
    b2 = np.asarray(b2, dtype=np.float32)
    W3 = np.asarray(W3, dtype=np.float32)
    b3 = np.asarray(b3, dtype=np.float32)

    in_maps = []
    for c in range(N_CORES):
        idx = 8 * np.arange(KPC) + c
        mask = (np.arange(N)[None, :] >= idx[:, None]).astype(np.float32)
        in_maps.append({
            "x": x,
            "xr": np.ascontiguousarray(x[idx]),
            "mask": mask,
            "w1a": np.ascontiguousarray(W1[0]),
            "w1b": np.ascontiguousarray(W1[1]),
            "b1": b1,
            "w2": W2,
            "b2": b2,
            "w3": np.ascontiguousarray(W3[:, 0]),
            "b3": b3,
        })

    res = run_bass_kernel_spmd(nc, in_maps, list(range(N_CORES)))
    out = np.zeros((N, N), dtype=np.float32)
    for c in range(N_CORES):
        out += res.results[c]["g"]
    return out


# revision 7
# speedup vs baseline: 3.1474x; 3.1474x over previous
"""Trainium2 Bass kernel for nn_NeuroKernel_5884105195994 (dense_mlp).

Computes: K = triu(MLP3(pairs(x))), output = K.T @ K, with
pairs[i,j] = (x[i], x[j]), MLP3 = (2->1024 relu) -> (1024->128 relu) -> (128->1).

Strategy (8 NeuronCores, SPMD):
  - Row i of the [512, 512] pair grid goes to core c = i % 8, local rank
    k = i // 8 (64 rows per core). Since only columns j >= i are needed
    (triu), row rank k computes the window [8k, 512) — identical on every
    core — and a per-core {0,1} mask input kills the j < i remainder, so
    the compiled program is core-independent (true SPMD).
  - Layer 1 has contraction dim 2, so it is NOT a matmul on device:
    h1[h, j] = relu(x_j * W1[1,h] + (x_i * W1[0,h] + b1[h])) is a single
    per-partition-scalar op (DVE tensor_scalar add+max / ACT relu-with-bias)
    over precomputed bT[h, j] = x_j * W1[1,h] tiles.
  - Layer 2 is the real work: h on partitions (8 chunks of 128), pair
    window on the free dim, accumulating W2-chunk matmuls into PSUM.
  - Rows are packed in complementary pairs {k, 64-k} so every matmul /
    elementwise op runs at free-dim 512 (window widths 512-8k and 8k).
  - Layer 3 is one [128,1]x[128,W] matmul per group; the [1, W] result is
    DMA'd into K_sb[64, 512] (row-major K block).
  - Each core computes its partial G = K_blk.T @ K_blk (contraction over
    its 64 rows, 4 matmuls) on the TensorEngine; the host sums 8 partials.
"""

import sys

if "/opt/trn_rl_repo" not in sys.path:
    sys.path.insert(0, "/opt/trn_rl_repo")

import numpy as np

N = 512
N_CORES = 8
KPC = N // N_CORES  # 64 rows per core
H1 = 1024
H2 = 128
NCHUNK = H1 // 128  # 8 h-chunks
# Per-h1-chunk engine assignment: v=VectorE, s=ScalarE(ACT), g=GpSimd
CHUNK_ENG = "vvvvvsgg"
F32R = True  # run matmuls as float32r (1 cycle/row vs 4 for plain fp32)


def _groups():
    """Row groups per core: list of lists of (k, col_off, j0, w)."""
    gs = [[(0, 0, 0, 512)]]
    for m in range(1, 32):
        w_a = 512 - 8 * m
        gs.append([(m, 0, 8 * m, w_a), (64 - m, w_a, 512 - 8 * m, 8 * m)])
    gs.append([(32, 0, 256, 256)])
    return gs


GROUPS = _groups()

_cached_nc = None


def _build():
    import concourse.bass as bass
    import concourse.mybir as mybir
    import concourse.tile as tile
    from concourse import bacc

    fp32 = mybir.dt.float32
    Alu = mybir.AluOpType
    Act = mybir.ActivationFunctionType

    # PE operand dtype: float32r streams 1 row/cycle (vs 4 for float32) at
    # free-dim >= 256. The BIR verifier requires every tensor consumed by an
    # fp32r matmul to be *produced* with fp32r rounding, so all matmul-operand
    # tiles are allocated as float32r and written by their producers directly.
    fmm = mybir.dt.float32r if F32R else mybir.dt.float32

    nc = bacc.Bacc(None, target_bir_lowering=False)

    def bcast(ap, parts):
        # Prepend a stride-0 partition dim: replicate a DRAM vector across
        # `parts` SBUF partitions in one DMA.
        return bass.AP(tensor=ap.tensor, offset=ap.offset,
                       ap=[[0, parts]] + [list(d) for d in ap.ap])

    x_d = nc.dram_tensor("x", [N], fp32, kind="ExternalInput")
    xr_d = nc.dram_tensor("xr", [KPC], fp32, kind="ExternalInput")
    mask_d = nc.dram_tensor("mask", [KPC, N], fp32, kind="ExternalInput")
    w1a_d = nc.dram_tensor("w1a", [H1], fp32, kind="ExternalInput")
    w1b_d = nc.dram_tensor("w1b", [H1], fp32, kind="ExternalInput")
    b1_d = nc.dram_tensor("b1", [H1], fp32, kind="ExternalInput")
    w2_d = nc.dram_tensor("w2", [H1, H2], fp32, kind="ExternalInput")
    b2_d = nc.dram_tensor("b2", [H2], fp32, kind="ExternalInput")
    w3_d = nc.dram_tensor("w3", [H2], fp32, kind="ExternalInput")
    b3_d = nc.dram_tensor("b3", [1], fp32, kind="ExternalInput")
    g_d = nc.dram_tensor("g", [N, N], fp32, kind="ExternalOutput")

    with tile.TileContext(nc) as tc:
        with (
            tc.tile_pool(name="const", bufs=1) as const,
            tc.tile_pool(name="h1p", bufs=4) as h1p,
            tc.tile_pool(name="h2p", bufs=4) as h2p,
            tc.tile_pool(name="kout", bufs=1) as kout,
            tc.tile_pool(name="psum", bufs=4, space="PSUM") as psum,
            tc.tile_pool(name="kpsum", bufs=3, space="PSUM") as kpsum,
        ):
            # ---- constants / precompute ----
            w1a_t = const.tile([128, NCHUNK], fp32)
            nc.sync.dma_start(w1a_t[:], w1a_d[:].rearrange("(c p) -> p c", p=128))
            w1b_t = const.tile([128, NCHUNK], fp32)
            nc.sync.dma_start(w1b_t[:], w1b_d[:].rearrange("(c p) -> p c", p=128))
            b1_t = const.tile([128, NCHUNK], fp32)
            nc.sync.dma_start(b1_t[:], b1_d[:].rearrange("(c p) -> p c", p=128))
            b2_t = const.tile([128, 1], fp32)
            nc.sync.dma_start(b2_t[:], b2_d[:].rearrange("(p one) -> p one", one=1))
            w3_f = const.tile([128, 1], fp32)
            nc.sync.dma_start(w3_f[:], w3_d[:].rearrange("(p one) -> p one", one=1))
            w3_t = const.tile([128, 1], fmm)
            nc.vector.tensor_copy(w3_t[:], w3_f[:])
            b3_t = const.tile([KPC, 1], fp32)
            nc.sync.dma_start(b3_t[:], bcast(b3_d[:], KPC))
            mask_t = const.tile([KPC, N], fp32)
            nc.sync.dma_start(mask_t[:], mask_d[:])
            w2_f = const.tile([128, NCHUNK, H2], fp32)
            nc.sync.dma_start(w2_f[:], w2_d[:].rearrange("(c p) k -> p c k", p=128))
            w2_t = const.tile([128, NCHUNK, H2], fmm)
            nc.vector.tensor_copy(w2_t[:], w2_f[:])

            xr_b = const.tile([128, KPC], fp32)
            nc.sync.dma_start(xr_b[:], bcast(xr_d[:], 128))
            xc_b = const.tile([128, N], fp32)
            nc.sync.dma_start(xc_b[:], bcast(x_d[:], 128))

            # bias[h, k] = x_row[k] * W1[0, h] + b1[h]
            bias_t = const.tile([128, NCHUNK, KPC], fp32)
            for cc in range(NCHUNK):
                nc.vector.tensor_scalar(
                    bias_t[:, cc, :], xr_b[:],
                    w1a_t[:, cc : cc + 1], b1_t[:, cc : cc + 1],
                    Alu.mult, Alu.add,
                )
            # bT[h, j] = x_col[j] * W1[1, h]
            bT_t = const.tile([128, NCHUNK, N], fp32)
            for cc in range(NCHUNK):
                nc.vector.tensor_scalar(
                    bT_t[:, cc, :], xc_b[:], w1b_t[:, cc : cc + 1], None, Alu.mult
                )

            k_sb = kout.tile([KPC, N], fp32)
            nc.gpsimd.memset(k_sb[:], 0.0)

            # ---- main loop over row groups ----
            for rows in GROUPS:
                wg = sum(r[3] for r in rows)
                h1 = h1p.tile([128, NCHUNK, wg], fmm, tag="h1")
                for cc in range(NCHUNK):
                    for (k, off, j0, w) in rows:
                        out_ap = h1[:, cc, off : off + w]
                        in_ap = bT_t[:, cc, j0 : j0 + w]
                        bias_ap = bias_t[:, cc, k : k + 1]
                        eng = CHUNK_ENG[cc]
                        if eng == "v":
                            nc.vector.tensor_scalar(
                                out_ap, in_ap, bias_ap, 0.0, Alu.add, Alu.max
                            )
                        elif eng == "g":
                            nc.gpsimd.tensor_scalar(
                                out_ap, in_ap, bias_ap, 0.0, Alu.add, Alu.max
                            )
                        else:
                            nc.scalar.activation(out_ap, in_ap, Act.Relu, bias=bias_ap)
                ps = psum.tile([128, wg], fp32, tag="ps")
                for cc in range(NCHUNK):
                    nc.tensor.matmul(
                        ps[:, :wg], w2_t[:, cc, :], h1[:, cc, :wg],
                        start=(cc == 0), stop=(cc == NCHUNK - 1),
                    )
                h2 = h2p.tile([128, wg], fmm, tag="h2")
                nc.scalar.activation(h2[:], ps[:, :wg], Act.Relu, bias=b2_t[:, 0:1])
                kps = kpsum.tile([1, wg], fp32, tag="kps")
                nc.tensor.matmul(kps[0:1, :wg], w3_t[:, 0:1], h2[:, :wg],
                                 start=True, stop=True)
                kst = h2p.tile([1, wg], fp32, tag="kst")
                nc.scalar.activation(kst[0:1, :wg], kps[0:1, :wg], Act.Copy)
                for (k, off, j0, w) in rows:
                    nc.sync.dma_start(
                        k_sb[k : k + 1, j0 : j0 + w], kst[0:1, off : off + w]
                    )

            # ---- K = (K + b3) * mask, then G = K.T @ K (64-row partial) ----
            km = kout.tile([KPC, N], fp32)
            nc.vector.tensor_scalar(km[:], k_sb[:], b3_t[:, 0:1], None, Alu.add)
            km2 = kout.tile([KPC, N], fmm)
            nc.vector.tensor_tensor(km2[:], km[:], mask_t[:], Alu.mult)
            for a in range(4):
                gps = psum.tile([128, N], fp32, tag="ps")
                nc.tensor.matmul(
                    gps[:], km2[:, 128 * a : 128 * (a + 1)], km2[:],
                    start=True, stop=True,
                )
                gsb = h1p.tile([128, N], fp32, tag="gsb")
                nc.vector.tensor_copy(gsb[:], gps[:])
                nc.sync.dma_start(g_d[128 * a : 128 * (a + 1), :], gsb[:])

    nc.compile()
    return nc


def kernel(x, W1, b1, W2, b2, W3, b3):
    global _cached_nc
    if _cached_nc is None:
        _cached_nc = _build()
    nc = _cached_nc

    from concourse.bass_utils import run_bass_kernel_spmd

    x = np.asarray(x, dtype=np.float32)
    W1 = np.asarray(W1, dtype=np.float32)
    b1 = np.asarray(b1, dtype=np.float32)
    W2 = np.asarray(W2, dtype=np.float32)
    b2 = np.asarray(b2, dtype=np.float32)
    W3 = np.asarray(W3, dtype=np.float32)
    b3 = np.asarray(b3, dtype=np.float32)

    in_maps = []
    for c in range(N_CORES):
        idx = 8 * np.arange(KPC) + c
        mask = (np.arange(N)[None, :] >= idx[:, None]).astype(np.float32)
        in_maps.append({
            "x": x,
            "xr": np.ascontiguousarray(x[idx]),
            "mask": mask,
            "w1a": np.ascontiguousarray(W1[0]),
            "w1b": np.ascontiguousarray(W1[1]),
            "b1": b1,
            "w2": W2,
            "b2": b2,
            "w3": np.ascontiguousarray(W3[:, 0]),
            "b3": b3,
        })

    res = run_bass_kernel_spmd(nc, in_maps, list(range(N_CORES)))
    out = np.zeros((N, N), dtype=np.float32)
    for c in range(N_CORES):
        out += res.results[c]["g"]
    return out
